# revision 44
# baseline (speedup 1.0000x reference)
"""Trainium2 Bass kernel for nn_AttentionAggregator3d.

Math (per batch b):
    zmf = zm.reshape(CM, N)                     # N = D*W*H = 4096 tokens
    q = Wq @ zmf + bq ; k = Wk @ zmf + bk       # (16, N)
    v = Wv @ zmf + bv                           # (128, N)
    A = softmax_n(q^T k)                        # (N, N), softmax over keys n
    out = v @ A^T ; result = zc + gamma * out

Key transformations used by the kernel:
  * logits chunk L[k, m] = zm[:, key_k] . U[:, m] with
    U = (Wk^T Wq) @ zm[:, queryblock] computed ON DEVICE once (1024 cols)
    -- the key-side "T = G zm" array over all 4096 keys is gone, the
    in-loop logits weights are the raw zm key chunks.
  * bq/bk only affect softmax through the per-key term r[n] = (Wk^T bq).zm[:,n]
    (per-query terms cancel in softmax); precomputed on host and applied as
    a per-partition exp bias.
  * Sharding: 8 cores = batch (2) x query-block (4, 1024 queries each). Each
    core sees its batch's zm rotated so its query block sits at columns 0:1024
    (softmax/PV sum over all keys, so key order is irrelevant).
  * zm ships twice from host: f32 (logits/U operands, no expansion casts)
    and bf16 (value-projection weights), both piece-major contiguous in DRAM
    so every input DMA is a cheap contiguous descriptor.
  * Layout: exp'd scores E^T are kept (keys on partitions, queries free) so
    the PV matmul contracts over keys on the PE in float32r. Softmax
    denominators: DVE accumulates full [128,1024] chunk adds for 23 chunks,
    GpSimd for 8 (j%4==1), chunk 31 + the accumulator folds run as PE
    matmuls into a [2,512] PSUM tile using e0/e1 selector weights (half 0
    lands in partition 0, half 1 in partition 1; the unused partition gets
    +0). 1/s: DVE reciprocal for half 0 in parallel with ACT ln/exp for
    half 1. gamma is folded into the [1,128] broadcast matmul weights, so
    rb = gamma/s lands directly; the tail is multiply + residual-add + DMA
    per quarter with DMAs alternating sync/scalar queues.

Perf notes from previous sessions (things that measure SLOWER, do not retry):
  * The all-f32r K=128 matmul mix is load-bearing: K=17 q/k-style logits,
    fp16/bf16 E tiles, or zero-valued PE weights all keep the PE at a
    ~1 ns/row p-state instead of ~0.45.
  * 1024-wide matmul outputs are rejected by the ISA (1 PSUM bank max).
  * Replacing the tail scalar_tensor_tensor with tensor_tensor, or moving
    the final residual add to GpSimd, measured ~+12us on the graded metric.
  * The device exec-time metric is bimodal run-to-run (~71 vs ~85) with the
    mode sticky per NEFF load; compare variants only across fresh loads.
  * scalar_tensor_tensor is not supported on the Pool engine (ISA check).
"""

import os
import sys
import types

import ml_dtypes
import numpy as np

import concourse.bacc as bacc_mod
import concourse.tile as tile
from concourse import mybir
from concourse.bass_utils import run_bass_kernel_spmd

B, CC, CM, P = 2, 128, 128, 16
N = 16 * 16 * 16          # 4096 tokens
MBLK = N // 4             # 1024 queries per core
NCORES = 8
NCHUNK = N // 128         # 32 key chunks of 128

F32 = mybir.dt.float32
F32R = mybir.dt.float32r
BF16 = mybir.dt.bfloat16
AF = mybir.ActivationFunctionType
ALU = mybir.AluOpType

LAST_RESULTS = None  # BassKernelResults of the most recent run (for test.py)


def _ensure_ntff_hook() -> bool:
    try:
        import antenv.axon_hooks  # noqa: F401

        return True
    except ImportError:
        pass
    try:
        import antenv
        from trn_agent_boot.trn_boot import _ntff_profile_via_ctypes

        hook = _ntff_profile_via_ctypes("/opt/axon/libaxon_pjrt.so")
        mod = types.ModuleType("antenv.axon_hooks")
        mod.get_axon_ntff_profile_hook = lambda: hook
        mod.set_axon_ntff_profile_hook = lambda h: None
        sys.modules["antenv.axon_hooks"] = mod
        antenv.axon_hooks = mod
        return hook is not None
    except Exception:
        return False


_orig_gat = bacc_mod.get_activation_tables
_COMBINED_SET = "natural_log_exp_and_others"


def _patched_gat(arch):
    tabs = _orig_gat(arch)
    if _COMBINED_SET in tabs:
        for name, fns in tabs.items():
            if name != _COMBINED_SET:
                fns.discard(AF.Exp)
                fns.discard(AF.Ln)
    return tabs


bacc_mod.get_activation_tables = _patched_gat


def _build(use_qk_bias: bool):
    nc = bacc_mod.Bacc(
        "TRN2",
        target_bir_lowering=False,
        debug=False,
        num_devices=NCORES,
    )

    # zm ships piece-major so every DMA below is DRAM-contiguous
    zm_d = nc.dram_tensor(
        "zm", (8, CM, 512), mybir.dt.float16, kind="ExternalInput"
    ).ap()
    zmh_d = nc.dram_tensor("zmh", (2, CM, 512), F32R, kind="ExternalInput").ap()
    zc_d = nc.dram_tensor("zc", (CC, MBLK), F32, kind="ExternalInput").ap()
    gq_d = nc.dram_tensor("gq", (CM, CM), F32R, kind="ExternalInput").ap()
    wvt_d = nc.dram_tensor("wvt", (CM, CC), BF16, kind="ExternalInput").ap()
    gam_d = nc.dram_tensor("gam", (33, CC), F32R, kind="ExternalInput").ap()
    adv_d = nc.dram_tensor("adv", (CC, 1), F32, kind="ExternalInput").ap()
    onesc_d = nc.dram_tensor("onesc", (128, 1), F32R, kind="ExternalInput").ap()
    if use_qk_bias:
        rn_d = nc.dram_tensor("rn", (128, NCHUNK), F32, kind="ExternalInput").ap()
    out_d = nc.dram_tensor("out", (CC, MBLK), F32, kind="ExternalOutput").ap()

    with tile.TileContext(nc) as tc:
        with (
            tc.tile_pool(name="consts", bufs=1) as consts,
            tc.tile_pool(name="epool", bufs=8) as epool,
            tc.tile_pool(name="lpool", bufs=2, space="PSUM") as lpool,
            tc.tile_pool(name="tpool", bufs=1, space="PSUM") as tpool,
            tc.tile_pool(name="opool", bufs=1, space="PSUM") as opool,
            tc.tile_pool(name="spool", bufs=1, space="PSUM") as spool,
        ):
            zm_sb = consts.tile([CM, N], F32R, tag="zm")
            zm16 = consts.tile([CM, N], mybir.dt.float16, tag="zm16")
            zmb_sb = consts.tile([CM, N], BF16, tag="zmb")
            u_sb = consts.tile([CM, MBLK], F32R, tag="u")
            vt_sb = consts.tile([128, N], F32R, tag="vt")  # chunk j at cols 128j
            zc_sb = consts.tile([CC, MBLK], F32, tag="zc")
            warm = consts.tile([1, 8], F32, tag="warm")
            gq_sb = consts.tile([CM, CM], F32R, tag="gq")
            wvt_sb = consts.tile([CM, CC], BF16, tag="wvt")
            gam_sb = consts.tile([33, CC], F32R, tag="gam")
            adv_sb = consts.tile([CC, 1], F32, tag="adv")
            ones_col = consts.tile([128, 1], F32R, tag="onesc")
            acc0 = consts.tile([128, 512], F32R, tag="acc0")
            acc = consts.tile([128, 512], F32R, tag="acc")
            accg = consts.tile([128, 512], F32R, tag="accg")
            rvec = consts.tile([1, MBLK], F32R, tag="rvec")
            rb_sb = consts.tile([128, MBLK], F32, tag="rb")
            lns = consts.tile([1, MBLK], F32, tag="lns")
            tmp_sb = consts.tile([CC, MBLK], F32, tag="tmp")
            out_sb = consts.tile([CC, MBLK], F32, tag="outsb")
            if use_qk_bias:
                rn_sb = consts.tile([128, NCHUNK], F32, tag="rn")

            # dummy exp at t=0: prefetches the Exp/Ln ACT table set while
            # the input DMAs stream
            nc.vector.memset(warm[:], 0.0)
            nc.scalar.activation(warm[:], warm[:], AF.Exp)

            # input DMAs: 4 queues, ordered by first use
            nc.sync.dma_start(zm_sb[:, 0:512], zmh_d[0])
            nc.scalar.dma_start(gq_sb[:], gq_d)
            nc.gpsimd.dma_start(zm_sb[:, 512:1024], zmh_d[1])
            nc.scalar.dma_start(wvt_sb[:], wvt_d)
            nc.scalar.dma_start(ones_col[:], onesc_d)
            nc.scalar.dma_start(gam_sb[:], gam_d)
            nc.scalar.dma_start(adv_sb[:], adv_d)
            nc.sync.dma_start(zm16[:, 0:512], zm_d[0])
            nc.gpsimd.dma_start(zm16[:, 512:1024], zm_d[1])
            nc.scalar.dma_start(zm16[:, 1024:1536], zm_d[2])
            nc.sync.dma_start(zm16[:, 1536:2048], zm_d[3])
            nc.gpsimd.dma_start(zm16[:, 2048:2560], zm_d[4])
            nc.scalar.dma_start(zm16[:, 2560:3072], zm_d[5])
            nc.sync.dma_start(zm16[:, 3072:3584], zm_d[6])
            nc.gpsimd.dma_start(zm16[:, 3584:4096], zm_d[7])
            if use_qk_bias:
                nc.gpsimd.dma_start(rn_sb[:], rn_d)
            nc.sync.dma_start(zc_sb[:], zc_d)

            # fp16 -> f32r expansion on the DVE (cols 0:1024 arrive as
            # f32 directly via zmh, so only the key-side pieces remain;
            # chunks 16/24 pieces are emitted in-loop just in time)
            nc.vector.tensor_copy(zm_sb[:, 1024:2048], zm16[:, 1024:2048])

            out_ps = opool.tile([CC, MBLK], F32, tag="out")
            s0_ps = spool.tile([1, 512], F32, tag="s0")
            s_half = [s0_ps, None]  # s_half[1] allocated from tpool late

            # U = (Wk^T Wq) zm_q : 2 x [128,512] matmuls; first piece split
            # 256-wide so the first logits matmul can fire earlier
            # the two U pieces go through the two lpool banks so the
            # second matmul overlaps the first PSUM->SBUF copy
            for c0, c1 in ((0, 512), (512, 1024)):
                ups = lpool.tile([128, MBLK], F32, tag="L")
                nc.tensor.matmul(
                    ups[0:128, 0:512],
                    gq_sb[:],
                    zm_sb[:, c0:c1],
                    start=True,
                    stop=True,
                )
                nc.scalar.copy(u_sb[:, c0:c1], ups[0:128, 0:512])

            def emit_vt_batch(i):
                # bf16 weights cast directly from the fp16 shipment
                # (bit-identical to casting the f32r copy; DVE 16-bit
                # fast mode)
                nc.vector.tensor_copy(
                    zmb_sb[:, i * 512 : (i + 1) * 512],
                    zm16[:, i * 512 : (i + 1) * 512],
                )
                vps = tpool.tile([128, 512], F32, tag="T")
                for k in range(4):
                    j = 4 * i + k
                    nc.tensor.matmul(
                        vps[:, 128 * k : 128 * (k + 1)],
                        zmb_sb[:, 128 * j : 128 * (j + 1)],
                        wvt_sb[:],
                        start=True,
                        stop=True,
                    )
                nc.vector.tensor_copy(vt_sb[:, i * 512 : (i + 1) * 512], vps[:])

            e_tiles = {}

            LAG = int(os.environ.get("BASS_PV_LAG", "3"))
            for j in range(NCHUNK + LAG):
                if j < NCHUNK:
                    if j in (3, 7):
                        c0 = 1024 * ((j - 3) // 4 + 2)
                        nc.vector.tensor_copy(
                            zm_sb[:, c0 : c0 + 1024], zm16[:, c0 : c0 + 1024]
                        )
                    if j % 4 == 2 and j // 4 + 1 <= 7:
                        emit_vt_batch(j // 4 + 1)
                    lps = lpool.tile([128, MBLK], F32, tag="L")
                    for h in range(2):
                        nc.tensor.matmul(
                            lps[:, h * 512 : (h + 1) * 512],
                            zm_sb[:, 128 * j : 128 * (j + 1)],
                            u_sb[:, h * 512 : (h + 1) * 512],
                            start=True,
                            stop=True,
                        )
                    ej = epool.tile([128, MBLK], F32R, tag="E")
                    bias = rn_sb[:, j : j + 1] if use_qk_bias else 0.0
                    nc.scalar.activation(ej[:], lps[:], AF.Exp, bias=bias)
                    e_tiles[j] = ej
                    if j == 0:
                        emit_vt_batch(0)
                if j >= LAG:
                    jj = j - LAG
                    ej = e_tiles.pop(jj)
                    # softmax denominator. half 0 (cols 0:512): PE ones-
                    # matmuls into the s0 PSUM tile for 2/3 of chunks,
                    # DVE SBUF accumulator acc0 for the rest. half 1
                    # (cols 512:1024): GpSimd accumulator 1/3, DVE acc
                    # 2/3. SBUF accumulators stop at jj==27 and chunks
                    # 28-31 go straight to the PE s tiles so the folds
                    # overlap the last chunks and the tail sees s almost
                    # immediately after the last exp.
                    if jj % 3 == 2 and jj <= 27:
                        if jj == 2:
                            nc.vector.tensor_copy(acc0[:], ej[:, 0:512])
                        else:
                            nc.vector.tensor_add(acc0[:], acc0[:], ej[:, 0:512])
                    else:
                        nc.tensor.matmul(
                            s_half[0][0:1, :],
                            ones_col[:],
                            ej[:, 0:512],
                            start=(jj == 0),
                            stop=(jj == NCHUNK - 1),
                            skip_group_check=True,
                        )
                    if jj == 29:
                        # accg complete (last gpsimd add jj==27); fold
                        # into s1 (claims the tpool bank, free since the
                        # last V batch)
                        s_half[1] = tpool.tile(
                            [1, 512], F32, tag="T", name="s1_ps"
                        )
                        nc.tensor.matmul(
                            s_half[1][0:1, :],
                            ones_col[:],
                            accg[:],
                            start=True,
                            stop=False,
                            skip_group_check=True,
                        )
                    if jj % 2 == 1 and jj <= 27:
                        if jj == 1:
                            nc.gpsimd.tensor_copy(accg[:], ej[:, 512:1024])
                        else:
                            nc.gpsimd.tensor_add(accg[:], accg[:], ej[:, 512:1024])
                    else:
                        if jj == 0:
                            nc.vector.tensor_copy(acc[:], ej[:, 512:1024])
                        else:
                            nc.vector.tensor_add(acc[:], acc[:], ej[:, 512:1024])
                    if jj == 28:
                        # acc0 complete (last DVE add jj==26)
                        nc.tensor.matmul(
                            s_half[0][0:1, :],
                            ones_col[:],
                            acc0[:],
                            start=False,
                            stop=False,
                            skip_group_check=True,
                        )
                    for h in range(2):
                        nc.tensor.matmul(
                            out_ps[:, h * 512 : (h + 1) * 512],
                            vt_sb[:, 128 * jj : (jj + 1) * 128],
                            ej[:, h * 512 : (h + 1) * 512],
                            start=(jj == 0),
                            stop=(jj == NCHUNK - 1),
                        )

            # acc complete (last DVE add was jj==31): fold it into s1
            nc.tensor.matmul(
                s_half[1][0:1, :],
                ones_col[:],
                acc[:],
                start=False,
                stop=True,
                skip_group_check=True,
            )

            # 1/s as exp(-ln s) on ACT; gamma folds into the broadcast
            # weights so rb = gamma / s lands directly
            rbt = lpool.tile([128, MBLK], F32, tag="L")
            for h in range(2):
                sl = slice(h * 512, (h + 1) * 512)
                nc.scalar.activation(lns[:, sl], s_half[h][0:1, :], AF.Ln)
                nc.scalar.activation(
                    rvec[:, sl], lns[:, sl], AF.Exp, scale=-1.0
                )
                nc.tensor.matmul(
                    rbt[:, sl],
                    gam_sb[0:1, :],
                    rvec[:, sl],
                    start=True,
                    stop=True,
                    skip_group_check=True,
                )
                nc.scalar.copy(rb_sb[:, sl], rbt[:, sl])
            # quartered endgame: multiply + residual add + DMA, with the
            # output DMAs alternating between the sync and scalar queues
            for q in range(4):
                sl = slice(q * 256, (q + 1) * 256)
                nc.vector.tensor_tensor(
                    tmp_sb[:, sl], out_ps[:, sl], rb_sb[:, sl], op=ALU.mult
                )
                nc.vector.scalar_tensor_tensor(
                    out_sb[:, sl],
                    tmp_sb[:, sl],
                    adv_sb[:, 0:1],
                    zc_sb[:, sl],
                    op0=ALU.add,
                    op1=ALU.add,
                )
                dq = nc.sync if q % 2 == 0 else nc.scalar
                dq.dma_start(out_d[:, sl], out_sb[:, sl])

    nc.compile()
    return nc


_CACHE = {}


def _get_program(use_qk_bias: bool):
    if use_qk_bias not in _CACHE:
        _CACHE[use_qk_bias] = _build(use_qk_bias)
    return _CACHE[use_qk_bias]


def kernel(zc, zm, Wq, bq, Wk, bk, Wv, bv, gamma):
    global LAST_RESULTS
    zc = np.ascontiguousarray(zc, dtype=np.float32)
    zm = np.ascontiguousarray(zm, dtype=np.float32)
    zmf = zm.reshape(B, CM, N)
    zcf = zc.reshape(B, CC, N)

    Wq = np.asarray(Wq, dtype=np.float32)
    Wk = np.asarray(Wk, dtype=np.float32)
    Wv = np.asarray(Wv, dtype=np.float32)
    gq = (Wq.astype(np.float64).T @ Wk.astype(np.float64)).astype(np.float32)
    wvt = np.ascontiguousarray(Wv.T).astype(ml_dtypes.bfloat16)
    gamma_v = np.float32(np.asarray(gamma).reshape(-1)[0])
    gam_arr = np.zeros((33, CC), dtype=np.float32)
    gam_arr[0, :] = gamma_v
    gam_arr[32, :] = gamma_v
    gam_arr = np.ascontiguousarray(gam_arr)
    adv_arr = (gamma_v * np.asarray(bv, dtype=np.float32)).reshape(CC, 1)
    adv_arr = np.ascontiguousarray(adv_arr)
    onesc = np.ones((128, 1), dtype=np.float32)

    use_qk_bias = bool(np.any(bq)) or bool(np.any(bk))
    nc = _get_program(use_qk_bias)

    in_maps = []
    for c in range(NCORES):
        b, jblk = divmod(c, 4)
        zmr = np.roll(zmf[b], -MBLK * jblk, axis=1)
        m = {
            "zm": np.ascontiguousarray(
                zmr.astype(np.float16).reshape(CM, 8, 512).transpose(1, 0, 2)
            ),
            "zmh": np.ascontiguousarray(
                zmr[:, 0:1024].reshape(CM, 2, 512).transpose(1, 0, 2)
            ),
            "zc": np.ascontiguousarray(zcf[b][:, MBLK * jblk : MBLK * (jblk + 1)]),
            "gq": gq,
            "wvt": wvt,
            "gam": gam_arr,
            "adv": adv_arr,
            "onesc": onesc,
        }
        if use_qk_bias:
            u = (Wk.T @ np.asarray(bq, dtype=np.float32)).astype(np.float32)
            rnfull = u @ zmr  # (N,) per key
            m["rn"] = np.ascontiguousarray(
                rnfull.reshape(NCHUNK, 128).T.astype(np.float32)
            )
        in_maps.append(m)

    trace = bool(int(os.environ.get("BASS_KERNEL_TRACE", "0")))
    if trace and not _ensure_ntff_hook():
        trace = False
    res = run_bass_kernel_spmd(
        nc,
        in_maps,
        core_ids=list(range(NCORES)),
        trace=trace,
    )
    LAST_RESULTS = res

    out = np.empty((B, CC, N), dtype=np.float32)
    for c in range(NCORES):
        b, jblk = divmod(c, 4)
        out[b][:, MBLK * jblk : MBLK * (jblk + 1)] = res.results[c]["out"]
    return out.reshape(zc.shape)


# revision 45
# speedup vs baseline: 1.0480x; 1.0480x over previous
"""Trainium2 Bass kernel for nn_AttentionAggregator3d.

Math (per batch b):
    zmf = zm.reshape(CM, N)                     # N = D*W*H = 4096 tokens
    q = Wq @ zmf + bq ; k = Wk @ zmf + bk       # (16, N)
    v = Wv @ zmf + bv                           # (128, N)
    A = softmax_n(q^T k)                        # (N, N), softmax over keys n
    out = v @ A^T ; result = zc + gamma * out

Key transformations used by the kernel:
  * logits chunk L[k, m] = zm[:, key_k] . U[:, m] with
    U = (Wk^T Wq) @ zm[:, queryblock] computed ON DEVICE once (1024 cols)
    -- the key-side "T = G zm" array over all 4096 keys is gone, the
    in-loop logits weights are the raw zm key chunks.
  * bq/bk only affect softmax through the per-key term r[n] = (Wk^T bq).zm[:,n]
    (per-query terms cancel in softmax); precomputed on host and applied as
    a per-partition exp bias.
  * Sharding: 8 cores = batch (2) x query-block (4, 1024 queries each). Each
    core sees its batch's zm rotated so its query block sits at columns 0:1024
    (softmax/PV sum over all keys, so key order is irrelevant).
  * zm ships twice from host: f32 (logits/U operands, no expansion casts)
    and bf16 (value-projection weights), both piece-major contiguous in DRAM
    so every input DMA is a cheap contiguous descriptor.
  * Layout: exp'd scores E^T are kept (keys on partitions, queries free) so
    the PV matmul contracts over keys on the PE in float32r. Softmax
    denominators: DVE accumulates full [128,1024] chunk adds for 23 chunks,
    GpSimd for 8 (j%4==1), chunk 31 + the accumulator folds run as PE
    matmuls into a [2,512] PSUM tile using e0/e1 selector weights (half 0
    lands in partition 0, half 1 in partition 1; the unused partition gets
    +0). 1/s: DVE reciprocal for half 0 in parallel with ACT ln/exp for
    half 1. gamma is folded into the [1,128] broadcast matmul weights, so
    rb = gamma/s lands directly; the tail is multiply + residual-add + DMA
    per quarter with DMAs alternating sync/scalar queues.

Perf notes from previous sessions (things that measure SLOWER, do not retry):
  * The all-f32r K=128 matmul mix is load-bearing: K=17 q/k-style logits,
    fp16/bf16 E tiles, or zero-valued PE weights all keep the PE at a
    ~1 ns/row p-state instead of ~0.45.
  * 1024-wide matmul outputs are rejected by the ISA (1 PSUM bank max).
  * Replacing the tail scalar_tensor_tensor with tensor_tensor, or moving
    the final residual add to GpSimd, measured ~+12us on the graded metric.
  * The device exec-time metric is bimodal run-to-run (~71 vs ~85) with the
    mode sticky per NEFF load; compare variants only across fresh loads.
  * scalar_tensor_tensor is not supported on the Pool engine (ISA check).
"""

import os
import sys
import types

import ml_dtypes
import numpy as np

import concourse.bacc as bacc_mod
import concourse.tile as tile
from concourse import mybir
from concourse.bass_utils import run_bass_kernel_spmd

B, CC, CM, P = 2, 128, 128, 16
N = 16 * 16 * 16          # 4096 tokens
MBLK = N // 4             # 1024 queries per core
NCORES = 8
NCHUNK = N // 128         # 32 key chunks of 128

F32 = mybir.dt.float32
F32R = mybir.dt.float32r
BF16 = mybir.dt.bfloat16
AF = mybir.ActivationFunctionType
ALU = mybir.AluOpType

LAST_RESULTS = None  # BassKernelResults of the most recent run (for test.py)


def _ensure_ntff_hook() -> bool:
    try:
        import antenv.axon_hooks  # noqa: F401

        return True
    except ImportError:
        pass
    try:
        import antenv
        from trn_agent_boot.trn_boot import _ntff_profile_via_ctypes

        hook = _ntff_profile_via_ctypes("/opt/axon/libaxon_pjrt.so")
        mod = types.ModuleType("antenv.axon_hooks")
        mod.get_axon_ntff_profile_hook = lambda: hook
        mod.set_axon_ntff_profile_hook = lambda h: None
        sys.modules["antenv.axon_hooks"] = mod
        antenv.axon_hooks = mod
        return hook is not None
    except Exception:
        return False


_orig_gat = bacc_mod.get_activation_tables
_COMBINED_SET = "natural_log_exp_and_others"


def _patched_gat(arch):
    tabs = _orig_gat(arch)
    if _COMBINED_SET in tabs:
        for name, fns in tabs.items():
            if name != _COMBINED_SET:
                fns.discard(AF.Exp)
                fns.discard(AF.Ln)
    return tabs


bacc_mod.get_activation_tables = _patched_gat


def _build(use_qk_bias: bool):
    nc = bacc_mod.Bacc(
        "TRN2",
        target_bir_lowering=False,
        debug=False,
        num_devices=NCORES,
    )

    # zm ships piece-major so every DMA below is DRAM-contiguous
    zm_d = nc.dram_tensor(
        "zm", (8, CM, 512), mybir.dt.float16, kind="ExternalInput"
    ).ap()
    zc_d = nc.dram_tensor("zc", (CC, MBLK), F32, kind="ExternalInput").ap()
    gq_d = nc.dram_tensor("gq", (CM, CM), F32R, kind="ExternalInput").ap()
    wvt_d = nc.dram_tensor("wvt", (CM, CC), BF16, kind="ExternalInput").ap()
    gam_d = nc.dram_tensor("gam", (33, CC), F32R, kind="ExternalInput").ap()
    adv_d = nc.dram_tensor("adv", (CC, 1), F32, kind="ExternalInput").ap()
    onesc_d = nc.dram_tensor("onesc", (128, 1), F32R, kind="ExternalInput").ap()
    if use_qk_bias:
        rn_d = nc.dram_tensor("rn", (128, NCHUNK), F32, kind="ExternalInput").ap()
    out_d = nc.dram_tensor("out", (CC, MBLK), F32, kind="ExternalOutput").ap()

    with tile.TileContext(nc) as tc:
        with (
            tc.tile_pool(name="consts", bufs=1) as consts,
            tc.tile_pool(name="epool", bufs=8) as epool,
            tc.tile_pool(name="lpool", bufs=2, space="PSUM") as lpool,
            tc.tile_pool(name="tpool", bufs=1, space="PSUM") as tpool,
            tc.tile_pool(name="opool", bufs=1, space="PSUM") as opool,
            tc.tile_pool(name="spool", bufs=1, space="PSUM") as spool,
        ):
            zm_sb = consts.tile([CM, N], F32R, tag="zm")
            zm16 = consts.tile([CM, N], mybir.dt.float16, tag="zm16")
            zmb_sb = consts.tile([CM, N], BF16, tag="zmb")
            u_sb = consts.tile([CM, MBLK], F32R, tag="u")
            vt_sb = consts.tile([128, N], F32R, tag="vt")  # chunk j at cols 128j
            zc_sb = consts.tile([CC, MBLK], F32, tag="zc")
            warm = consts.tile([1, 8], F32, tag="warm")
            gq_sb = consts.tile([CM, CM], F32R, tag="gq")
            wvt_sb = consts.tile([CM, CC], BF16, tag="wvt")
            gam_sb = consts.tile([33, CC], F32R, tag="gam")
            adv_sb = consts.tile([CC, 1], F32, tag="adv")
            ones_col = consts.tile([128, 1], F32R, tag="onesc")
            acc0 = consts.tile([128, 512], F32R, tag="acc0")
            acc = consts.tile([128, 512], F32R, tag="acc")
            accg = consts.tile([128, 512], F32R, tag="accg")
            rvec = consts.tile([1, MBLK], F32R, tag="rvec")
            rb_sb = consts.tile([128, MBLK], F32, tag="rb")
            lns = consts.tile([1, MBLK], F32, tag="lns")
            tmp_sb = consts.tile([CC, MBLK], F32, tag="tmp")
            out_sb = consts.tile([CC, MBLK], F32, tag="outsb")
            if use_qk_bias:
                rn_sb = consts.tile([128, NCHUNK], F32, tag="rn")

            # dummy exp at t=0: prefetches the Exp/Ln ACT table set while
            # the input DMAs stream
            nc.vector.memset(warm[:], 0.0)
            nc.scalar.activation(warm[:], warm[:], AF.Exp)

            # input DMAs: 4 queues, ordered by first use
            nc.sync.dma_start(zm16[:, 0:512], zm_d[0])
            nc.scalar.dma_start(gq_sb[:], gq_d)
            nc.gpsimd.dma_start(zm16[:, 512:1024], zm_d[1])
            nc.scalar.dma_start(wvt_sb[:], wvt_d)
            nc.scalar.dma_start(ones_col[:], onesc_d)
            nc.scalar.dma_start(gam_sb[:], gam_d)
            nc.scalar.dma_start(adv_sb[:], adv_d)
            nc.sync.dma_start(zm16[:, 1024:1536], zm_d[2])
            nc.gpsimd.dma_start(zm16[:, 1536:2048], zm_d[3])
            nc.scalar.dma_start(zm16[:, 2048:2560], zm_d[4])
            nc.sync.dma_start(zm16[:, 2560:3072], zm_d[5])
            nc.gpsimd.dma_start(zm16[:, 3072:3584], zm_d[6])
            nc.scalar.dma_start(zm16[:, 3584:4096], zm_d[7])
            if use_qk_bias:
                nc.gpsimd.dma_start(rn_sb[:], rn_d)
            nc.sync.dma_start(zc_sb[:], zc_d)

            # fp16 -> f32r expansion on the DVE, piece by piece as the
            # DMAs land (single engine: concurrent casters on the same
            # tiles measured ~3x slower from SBUF contention). Pieces
            # 0-3 cover U + key chunks 0-7 and run now; the rest are
            # emitted inside the loop just in time (chunks 8/16/24) so
            # they don't delay the vt-batch casts in the DVE queue.
            for c0, c1 in ((0, 256), (256, 512), (512, 768), (768, 1024)):
                nc.vector.tensor_copy(zm_sb[:, c0:c1], zm16[:, c0:c1])

            out_ps = opool.tile([CC, MBLK], F32, tag="out")
            s0_ps = spool.tile([1, 512], F32, tag="s0")
            s_half = [s0_ps, None]  # s_half[1] allocated from tpool late

            # U = (Wk^T Wq) zm_q : 2 x [128,512] matmuls; first piece split
            # 256-wide so the first logits matmul can fire earlier
            # the two U pieces go through the two lpool banks so the
            # second matmul overlaps the first PSUM->SBUF copy
            for c0, c1 in ((0, 512), (512, 1024)):
                ups = lpool.tile([128, MBLK], F32, tag="L")
                nc.tensor.matmul(
                    ups[0:128, 0:512],
                    gq_sb[:],
                    zm_sb[:, c0:c1],
                    start=True,
                    stop=True,
                )
                nc.scalar.copy(u_sb[:, c0:c1], ups[0:128, 0:512])

            def emit_vt_batch(i):
                # bf16 weights cast directly from the fp16 shipment
                # (bit-identical to casting the f32r copy; DVE 16-bit
                # fast mode)
                nc.vector.tensor_copy(
                    zmb_sb[:, i * 512 : (i + 1) * 512],
                    zm16[:, i * 512 : (i + 1) * 512],
                )
                vps = tpool.tile([128, 512], F32, tag="T")
                for k in range(4):
                    j = 4 * i + k
                    nc.tensor.matmul(
                        vps[:, 128 * k : 128 * (k + 1)],
                        zmb_sb[:, 128 * j : 128 * (j + 1)],
                        wvt_sb[:],
                        start=True,
                        stop=True,
                    )
                nc.vector.tensor_copy(vt_sb[:, i * 512 : (i + 1) * 512], vps[:])

            e_tiles = {}

            LAG = int(os.environ.get("BASS_PV_LAG", "3"))
            for j in range(NCHUNK + LAG):
                if j < NCHUNK:
                    if j in (3, 7, 11):
                        c0 = 1024 * ((j - 3) // 4 + 1)
                        nc.vector.tensor_copy(
                            zm_sb[:, c0 : c0 + 1024], zm16[:, c0 : c0 + 1024]
                        )
                    if j % 4 == 2 and j // 4 + 1 <= 7:
                        emit_vt_batch(j // 4 + 1)
                    lps = lpool.tile([128, MBLK], F32, tag="L")
                    for h in range(2):
                        nc.tensor.matmul(
                            lps[:, h * 512 : (h + 1) * 512],
                            zm_sb[:, 128 * j : 128 * (j + 1)],
                            u_sb[:, h * 512 : (h + 1) * 512],
                            start=True,
                            stop=True,
                        )
                    ej = epool.tile([128, MBLK], F32R, tag="E")
                    bias = rn_sb[:, j : j + 1] if use_qk_bias else 0.0
                    nc.scalar.activation(ej[:], lps[:], AF.Exp, bias=bias)
                    e_tiles[j] = ej
                    if j == 0:
                        emit_vt_batch(0)
                if j >= LAG:
                    jj = j - LAG
                    ej = e_tiles.pop(jj)
                    # softmax denominator. half 0 (cols 0:512): PE ones-
                    # matmuls into the s0 PSUM tile for 2/3 of chunks,
                    # DVE SBUF accumulator acc0 for the rest. half 1
                    # (cols 512:1024): GpSimd accumulator 1/3, DVE acc
                    # 2/3. SBUF accumulators stop at jj==27 and chunks
                    # 28-31 go straight to the PE s tiles so the folds
                    # overlap the last chunks and the tail sees s almost
                    # immediately after the last exp.
                    if jj % 3 == 2 and jj <= 27:
                        if jj == 2:
                            nc.vector.tensor_copy(acc0[:], ej[:, 0:512])
                        else:
                            nc.vector.tensor_add(acc0[:], acc0[:], ej[:, 0:512])
                    else:
                        nc.tensor.matmul(
                            s_half[0][0:1, :],
                            ones_col[:],
                            ej[:, 0:512],
                            start=(jj == 0),
                            stop=(jj == NCHUNK - 1),
                            skip_group_check=True,
                        )
                    if jj == 29:
                        # accg complete (last gpsimd add jj==27); fold
                        # into s1 (claims the tpool bank, free since the
                        # last V batch)
                        s_half[1] = tpool.tile(
                            [1, 512], F32, tag="T", name="s1_ps"
                        )
                        nc.tensor.matmul(
                            s_half[1][0:1, :],
                            ones_col[:],
                            accg[:],
                            start=True,
                            stop=False,
                            skip_group_check=True,
                        )
                    if jj % 2 == 1 and jj <= 27:
                        if jj == 1:
                            nc.gpsimd.tensor_copy(accg[:], ej[:, 512:1024])
                        else:
                            nc.gpsimd.tensor_add(accg[:], accg[:], ej[:, 512:1024])
                    else:
                        if jj == 0:
                            nc.vector.tensor_copy(acc[:], ej[:, 512:1024])
                        else:
                            nc.vector.tensor_add(acc[:], acc[:], ej[:, 512:1024])
                    if jj == 28:
                        # acc0 complete (last DVE add jj==26)
                        nc.tensor.matmul(
                            s_half[0][0:1, :],
                            ones_col[:],
                            acc0[:],
                            start=False,
                            stop=False,
                            skip_group_check=True,
                        )
                    for h in range(2):
                        nc.tensor.matmul(
                            out_ps[:, h * 512 : (h + 1) * 512],
                            vt_sb[:, 128 * jj : (jj + 1) * 128],
                            ej[:, h * 512 : (h + 1) * 512],
                            start=(jj == 0),
                            stop=(jj == NCHUNK - 1),
                        )

            # acc complete (last DVE add was jj==31): fold it into s1
            nc.tensor.matmul(
                s_half[1][0:1, :],
                ones_col[:],
                acc[:],
                start=False,
                stop=True,
                skip_group_check=True,
            )

            # 1/s as exp(-ln s) on ACT; gamma folds into the broadcast
            # weights so rb = gamma / s lands directly
            rbt = lpool.tile([128, MBLK], F32, tag="L")
            for h in range(2):
                sl = slice(h * 512, (h + 1) * 512)
                nc.scalar.activation(lns[:, sl], s_half[h][0:1, :], AF.Ln)
                nc.scalar.activation(
                    rvec[:, sl], lns[:, sl], AF.Exp, scale=-1.0
                )
                nc.tensor.matmul(
                    rbt[:, sl],
                    gam_sb[0:1, :],
                    rvec[:, sl],
                    start=True,
                    stop=True,
                    skip_group_check=True,
                )
                nc.scalar.copy(rb_sb[:, sl], rbt[:, sl])
            # quartered endgame: multiply + residual add + DMA, with the
            # output DMAs alternating between the sync and scalar queues
            for q in range(4):
                sl = slice(q * 256, (q + 1) * 256)
                nc.vector.tensor_tensor(
                    tmp_sb[:, sl], out_ps[:, sl], rb_sb[:, sl], op=ALU.mult
                )
                nc.vector.scalar_tensor_tensor(
                    out_sb[:, sl],
                    tmp_sb[:, sl],
                    adv_sb[:, 0:1],
                    zc_sb[:, sl],
                    op0=ALU.add,
                    op1=ALU.add,
                )
                dq = nc.sync if q % 2 == 0 else nc.scalar
                dq.dma_start(out_d[:, sl], out_sb[:, sl])

    nc.compile()
    return nc


_CACHE = {}


def _get_program(use_qk_bias: bool):
    if use_qk_bias not in _CACHE:
        _CACHE[use_qk_bias] = _build(use_qk_bias)
    return _CACHE[use_qk_bias]


def kernel(zc, zm, Wq, bq, Wk, bk, Wv, bv, gamma):
    global LAST_RESULTS
    zc = np.ascontiguousarray(zc, dtype=np.float32)
    zm = np.ascontiguousarray(zm, dtype=np.float32)
    zmf = zm.reshape(B, CM, N)
    zcf = zc.reshape(B, CC, N)

    Wq = np.asarray(Wq, dtype=np.float32)
    Wk = np.asarray(Wk, dtype=np.float32)
    Wv = np.asarray(Wv, dtype=np.float32)
    gq = (Wq.astype(np.float64).T @ Wk.astype(np.float64)).astype(np.float32)
    wvt = np.ascontiguousarray(Wv.T).astype(ml_dtypes.bfloat16)
    gamma_v = np.float32(np.asarray(gamma).reshape(-1)[0])
    gam_arr = np.zeros((33, CC), dtype=np.float32)
    gam_arr[0, :] = gamma_v
    gam_arr[32, :] = gamma_v
    gam_arr = np.ascontiguousarray(gam_arr)
    adv_arr = (gamma_v * np.asarray(bv, dtype=np.float32)).reshape(CC, 1)
    adv_arr = np.ascontiguousarray(adv_arr)
    onesc = np.ones((128, 1), dtype=np.float32)

    use_qk_bias = bool(np.any(bq)) or bool(np.any(bk))
    nc = _get_program(use_qk_bias)

    in_maps = []
    for c in range(NCORES):
        b, jblk = divmod(c, 4)
        zmr = np.roll(zmf[b], -MBLK * jblk, axis=1)
        m = {
            "zm": np.ascontiguousarray(
                zmr.astype(np.float16).reshape(CM, 8, 512).transpose(1, 0, 2)
            ),
            "zc": np.ascontiguousarray(zcf[b][:, MBLK * jblk : MBLK * (jblk + 1)]),
            "gq": gq,
            "wvt": wvt,
            "gam": gam_arr,
            "adv": adv_arr,
            "onesc": onesc,
        }
        if use_qk_bias:
            u = (Wk.T @ np.asarray(bq, dtype=np.float32)).astype(np.float32)
            rnfull = u @ zmr  # (N,) per key
            m["rn"] = np.ascontiguousarray(
                rnfull.reshape(NCHUNK, 128).T.astype(np.float32)
            )
        in_maps.append(m)

    trace = bool(int(os.environ.get("BASS_KERNEL_TRACE", "0")))
    if trace and not _ensure_ntff_hook():
        trace = False
    res = run_bass_kernel_spmd(
        nc,
        in_maps,
        core_ids=list(range(NCORES)),
        trace=trace,
    )
    LAST_RESULTS = res

    out = np.empty((B, CC, N), dtype=np.float32)
    for c in range(NCORES):
        b, jblk = divmod(c, 4)
        out[b][:, MBLK * jblk : MBLK * (jblk + 1)] = res.results[c]["out"]
    return out.reshape(zc.shape)
